# revision 12
# baseline (speedup 1.0000x reference)
"""DiffJPEG forward (16x3x512x512, quality=80) on 8 TRN2 NeuronCores.

Data-parallel over batch (2 images/core). The image is pre-blocked on the
host into a [128, 2048] fp16 layout per channel slice where each partition
column holds the 64 pixels of two vertically-adjacent 8x8 blocks. That makes
the full 2D DCT a single dense 128x128 stationary matmul (kron(I2, M (x) M)),
so the whole pipeline is:

  fwd : psum[oc]  = sum_ic  (W2 * 255*WYCC[oc,ic])^T @ x_ic        (9 fp16 matmuls)
  round: q[oc]    = fp16_cast(psum * rq[p] + (1536 + dc_adj[p]))   (ACT; the fp16
         cast at magnitude [1024,2048) has ulp=1, so the cast itself rounds
         t to the nearest integer, RNE — matching jnp.round)
  inv : psum[ro]  = sum_yin (W2 * q[p]*coef/255)^T @ q_yin         (7 fp16 matmuls)
  out : o[ro]     = psum + corr[p]                                  (DVE; corr folds
         the -1536 offset removal and the +128/255 bias)

Host post-processing un-blocks, upcasts and clips to [0,1]. Input is centered
(x - 0.5) on the host to halve fp16 quantization error; the DC shift this
introduces is folded into the round bias. Quant tables replicate the
reference's flattened-(b,c)<16 luma/chroma split, so tables vary per core.
All matmuls run fp16 (1 cyc/row); measured rel-L2 vs the f32 reference is
~6.5e-3, dominated by round()-flips from fp16 input/weight rounding.
"""

import numpy as np

import concourse.bass as bass
import concourse.mybir as mybir
import concourse.tile as tile
from concourse import bacc
from concourse.bass_utils import run_bass_kernel_spmd

N_CORES = 8
BS = 16
IMGS_PER_CORE = BS // N_CORES          # 2
SLICES = IMGS_PER_CORE * 3             # 6
MAGIC = 1536.0                         # fp16 round-to-nearest at ulp=1 for [1024,2048)

F16 = mybir.dt.float16
F32 = mybir.dt.float32
IDENT = mybir.ActivationFunctionType.Identity

_LUM = np.array([[16,11,10,16,24,40,51,61],[12,12,14,19,26,58,60,55],[14,13,16,24,40,57,69,56],[14,17,22,29,51,87,80,62],[18,22,37,56,68,109,103,77],[24,35,55,64,81,104,113,92],[49,64,78,87,103,121,120,101],[72,92,95,98,112,100,103,99]], np.float64)
_CHROM = np.array([[17,18,24,47,99,99,99,99],[18,21,26,66,99,99,99,99],[24,26,56,99,99,99,99,99],[47,66,99,99,99,99,99,99],[99,99,99,99,99,99,99,99],[99,99,99,99,99,99,99,99],[99,99,99,99,99,99,99,99],[99,99,99,99,99,99,99,99]], np.float64)
_WYCC = np.array([[0.299, 0.587, 0.114], [-0.1687, -0.3313, 0.5], [0.5, -0.4187, -0.0813]], np.float64)
_S4TERMS = [
    [(0, 1.0), (2, 1.402)],                       # r
    [(0, 1.0), (1, -0.34414), (2, -0.71414)],     # g
    [(0, 1.0), (1, 1.772)],                       # b
]
_NTERM = 7


def _dct_mat():
    k = np.arange(8)[:, None]
    n = np.arange(8)[None, :]
    norm = np.where(k == 0, np.sqrt(1.0 / 8.0), np.sqrt(2.0 / 8.0))
    return norm * np.cos(np.pi / 8.0 * (n + 0.5) * k)


def _qtables(quality):
    q = max(1, min(100, int(quality)))
    scale = 5000.0 / q if q < 50 else 200.0 - 2.0 * q
    tbs = np.stack([_LUM, _CHROM]) * np.float32(scale)
    return np.clip((tbs.astype(np.float32) + 50.0) / 100.0, 1.0, 255.0).astype(np.float64)


def _w2():
    K64 = np.kron(_dct_mat(), _dct_mat())    # [freq 8u+v, pix 8r+c]
    return np.kron(np.eye(2), K64)           # [128 freq, 128 pix]


def _fwd_weights():
    """fp16 lhsT [pix, freq] per (oc, ic), packed [128, 9*128]."""
    W2 = _w2()
    w = np.zeros((128, 9 * 128), np.float16)
    for oc in range(3):
        for ic in range(3):
            w[:, 128 * (3 * oc + ic):128 * (3 * oc + ic + 1)] = \
                (W2 * (255.0 * _WYCC[oc, ic])).T.astype(np.float16)
    return w


def _core_tables(quality, core, fwdw):
    """Per-core inverse weights + ACT scale/bias + inverse corr vectors."""
    W2 = _w2()
    qt = _qtables(quality)
    invw = np.zeros((128, 2 * _NTERM * 128), np.float16)
    scl = np.zeros((128, SLICES), np.float32)
    bia = np.zeros((128, SLICES), np.float32)
    cor = np.zeros((128, SLICES), np.float32)
    for im in range(IMGS_PER_CORE):
        q2 = []
        rq2 = []
        for ch in range(3):
            tab = qt[0] if (6 * core + 3 * im + ch) < 16 else qt[1]
            qv = np.concatenate([tab.reshape(64), tab.reshape(64)])
            q2.append(qv)
            rq2.append((1.0 / qv).astype(np.float32))
        for oc in range(3):
            scl[:, 3 * im + oc] = rq2[oc]
            # round bias: +MAGIC, Y level shift, and centering compensation
            dct_adj = np.zeros(128, np.float64)
            if oc == 0:
                dct_adj[0] += -1024.0
                dct_adj[64] += -1024.0
            for ic in range(3):
                w = fwdw[:, 128 * (3 * oc + ic):128 * (3 * oc + ic + 1)].astype(np.float64)
                dct_adj += 0.5 * w.sum(axis=0)   # A(0.5*ones) at each freq
            bia[:, 3 * im + oc] = (MAGIC + rq2[oc].astype(np.float64) * dct_adj).astype(np.float32)
        t = 0
        for ro in range(3):
            corr = np.zeros(128, np.float64)
            for (yin, coef) in _S4TERMS[ro]:
                st = (W2 * (q2[yin][:, None] * (coef / 255.0))).astype(np.float16)
                invw[:, 128 * (_NTERM * im + t):128 * (_NTERM * im + t + 1)] = st
                corr += -MAGIC * st.astype(np.float64).sum(axis=0)
                t += 1
            cor[:, 3 * im + ro] = (corr + 128.0 / 255.0).astype(np.float32)
    return invw, scl, bia, cor


def _block(x):
    """[n, 512, 512] f32 -> [n, 128, 2048] blocked fp16 (centered)."""
    n = x.shape[0]
    return np.ascontiguousarray(
        (x - np.float32(0.5)).reshape(n, 32, 2, 8, 64, 8)
        .transpose(0, 2, 3, 5, 1, 4).reshape(n, 128, 2048).astype(np.float16)
    )


def _unblock(y):
    """[n, 128, 2048] -> [n, 512, 512]."""
    n = y.shape[0]
    return y.reshape(n, 2, 8, 8, 32, 64).transpose(0, 4, 1, 2, 5, 3).reshape(n, 512, 512)


def _trace():
    nc = bacc.Bacc("TRN2", target_bir_lowering=False, debug=False)

    xin = nc.dram_tensor("xin", [SLICES, 128, 2048], F16, kind="ExternalInput").ap()
    fwdw_d = nc.dram_tensor("fwdw", [128, 9 * 128], F16, kind="ExternalInput").ap()
    invw_d = nc.dram_tensor("invw", [128, 2 * _NTERM * 128], F16, kind="ExternalInput").ap()
    # vec packs [scl | bia | cor] as [128, 18] f32
    vec_d = nc.dram_tensor("vec", [128, 3 * SLICES], F32, kind="ExternalInput").ap()
    xout = nc.dram_tensor("xout", [SLICES, 128, 2048], F16, kind="ExternalOutput").ap()

    with tile.TileContext(nc) as tc:
        with (
            tc.tile_pool(name="wts", bufs=1) as wp,
            tc.tile_pool(name="xp", bufs=1) as xp,
            tc.tile_pool(name="qp", bufs=1) as qp,
            tc.tile_pool(name="op", bufs=1) as op,
            tc.tile_pool(name="psA", bufs=4, space="PSUM") as psAp,
            tc.tile_pool(name="psB", bufs=4, space="PSUM") as psBp,
        ):
            fwdw = wp.tile([128, 9 * 128], F16, tag="fwdw")
            nc.gpsimd.dma_start(fwdw[:], fwdw_d)

            # PE p-state warmup: burn the ramp on dummy matmuls while the
            # first input chunks are still in flight.
            warm = wp.tile([128, 512], F16, tag="warm")
            nc.vector.memzero(warm[:])
            for _w in range(6):
                wps = psAp.tile([128, 512], F32, tag="psA", name="wps")
                nc.tensor.matmul(wps[:], warm[:, 0:128], warm[:], start=True, stop=True)

            # per-image inputs as one [128, 3*2048] tile; one DMA per
            # 512-column chunk covering all 3 channels (single HWDGE slot);
            # the very first chunk is split in two 256-col pieces for latency
            xt = [None] * IMGS_PER_CORE
            xt[0] = xp.tile([128, 3 * 2048], F16, tag="x0", name="x0")
            x0v = xt[0][:].rearrange("p (c n) -> p c n", c=3)
            nc.sync.dma_start(
                x0v[:, :, 0:256], xin[0:3].rearrange("c p n -> p c n")[:, :, 0:256]
            )
            nc.sync.dma_start(
                x0v[:, :, 256:512], xin[0:3].rearrange("c p n -> p c n")[:, :, 256:512]
            )
            vec = wp.tile([128, 3 * SLICES], F32, tag="vec")
            nc.sync.dma_start(vec[:], vec_d)
            for s in range(1, 4):
                nc.sync.dma_start(
                    x0v[:, :, 512 * s:512 * (s + 1)],
                    xin[0:3].rearrange("c p n -> p c n")[:, :, 512 * s:512 * (s + 1)],
                )
            invw = wp.tile([128, 2 * _NTERM * 128], F16, tag="invw")
            nc.gpsimd.dma_start(invw[:], invw_d)
            xt[1] = xp.tile([128, 3 * 2048], F16, tag="x1", name="x1")
            nc.sync.dma_start(
                xt[1][:].rearrange("p (c n) -> p c n", c=3),
                xin[3:6].rearrange("c p n -> p c n"),
            )

            qt_ = [[None] * 3 for _ in range(IMGS_PER_CORE)]
            ot_ = [None] * IMGS_PER_CORE

            def fwd(im, lo, hi):
                if lo == 0:
                    for c in range(3):
                        qt_[im][c] = qp.tile([128, 2048], F16, tag=f"q{im}_{c}", name=f"q{im}_{c}")
                for oc in range(3):
                    ps = psAp.tile([128, 512], F32, tag="psA")
                    n = hi - lo
                    for k in range(3):
                        nc.tensor.matmul(
                            ps[:, 0:n], fwdw[:, 128 * (3 * oc + k):128 * (3 * oc + k + 1)],
                            xt[im][:, 2048 * k + lo:2048 * k + hi],
                            start=(k == 0), stop=(k == 2),
                        )
                    sl = 3 * im + oc
                    nc.scalar.activation(
                        qt_[im][oc][:, lo:hi], ps[:, 0:n], IDENT,
                        bias=vec[:, SLICES + sl:SLICES + sl + 1],
                        scale=vec[:, sl:sl + 1],
                    )

            def inv(im, s, last=False):
                t = 0
                if s == 0:
                    ot_[im] = op.tile([128, 3 * 2048], F16, tag=f"o{im}", name=f"o{im}")
                for ro in range(3):
                    terms = _S4TERMS[ro]
                    ps = psBp.tile([128, 512], F32, tag="psB")
                    for ti, (yin, _) in enumerate(terms):
                        nc.tensor.matmul(
                            ps[:], invw[:, 128 * (_NTERM * im + t):128 * (_NTERM * im + t + 1)],
                            qt_[im][yin][:, 512 * s:512 * (s + 1)],
                            start=(ti == 0), stop=(ti == len(terms) - 1),
                        )
                        t += 1
                    sl = 3 * im + ro
                    dst = ot_[im][:, 2048 * ro + 512 * s:2048 * ro + 512 * (s + 1)]
                    cv = vec[:, 2 * SLICES + sl:2 * SLICES + sl + 1]
                    if last:
                        # spread the final chunk's writes across ACT and DVE and
                        # DMA each channel as soon as it lands to shorten the tail
                        if ro == 1:
                            nc.scalar.activation(dst, ps[:], IDENT, bias=cv, scale=1.0)
                        else:
                            nc.vector.tensor_scalar_add(dst, ps[:], cv)
                        nc.sync.dma_start(
                            xout[sl, :, 512 * s:512 * (s + 1)], dst,
                        )
                    else:
                        nc.vector.tensor_scalar_add(dst, ps[:], cv)
                if not last:
                    # one DMA for this chunk across all 3 output channels
                    nc.sync.dma_start(
                        xout[3 * im:3 * im + 3].rearrange("c p n -> p c n")[:, :, 512 * s:512 * (s + 1)],
                        ot_[im][:].rearrange("p (c n) -> p c n", c=3)[:, :, 512 * s:512 * (s + 1)],
                    )

            for im in range(IMGS_PER_CORE):
                if im == 0:
                    fwd(im, 0, 256)
                    fwd(im, 256, 512)
                else:
                    fwd(im, 0, 512)
                fwd(im, 512, 1024)
                inv(im, 0)
                fwd(im, 1024, 1536)
                inv(im, 1)
                fwd(im, 1536, 2048)
                inv(im, 2)
                inv(im, 3, last=(im == IMGS_PER_CORE - 1))
    nc.compile()
    return nc


_COMPILED = None


def _get_compiled():
    global _COMPILED
    if _COMPILED is None:
        _COMPILED = _trace()
    return _COMPILED


def kernel(img, quality):
    img = np.ascontiguousarray(np.asarray(img, np.float32))
    quality = int(np.asarray(quality))
    nc = _get_compiled()

    fwdw = _fwd_weights()
    in_maps = []
    for core in range(N_CORES):
        invw, scl, bia, cor = _core_tables(quality, core, fwdw)
        shard = img[IMGS_PER_CORE * core:IMGS_PER_CORE * (core + 1)].reshape(SLICES, 512, 512)
        in_maps.append({
            "xin": _block(shard), "fwdw": fwdw, "invw": invw,
            "vec": np.ascontiguousarray(np.concatenate([scl, bia, cor], axis=1)),
        })

    res = run_bass_kernel_spmd(nc, in_maps, core_ids=list(range(N_CORES)))
    out = np.stack([
        _unblock(res.results[c]["xout"].astype(np.float32)) for c in range(N_CORES)
    ])
    return np.clip(out.reshape(BS, 3, 512, 512), 0.0, 1.0)


if __name__ == "__main__":
    rng = np.random.default_rng(0)
    x = rng.random((BS, 3, 512, 512), dtype=np.float32)
    y = kernel(x, 80)
    print("kernel ran:", y.shape, y.dtype, float(y.min()), float(y.max()))


# revision 13
# speedup vs baseline: 1.0088x; 1.0088x over previous
"""DiffJPEG forward (16x3x512x512, quality=80) on 8 TRN2 NeuronCores.

Data-parallel over batch (2 images/core). The image is pre-blocked on the
host into a [128, 2048] fp16 layout per channel slice where each partition
column holds the 64 pixels of two vertically-adjacent 8x8 blocks. That makes
the full 2D DCT a single dense 128x128 stationary matmul (kron(I2, M (x) M)),
so the whole pipeline is:

  fwd : psum[oc]  = sum_ic  (W2 * 255*WYCC[oc,ic])^T @ x_ic        (9 fp16 matmuls)
  round: q[oc]    = fp16_cast(psum * rq[p] + (1536 + dc_adj[p]))   (ACT; the fp16
         cast at magnitude [1024,2048) has ulp=1, so the cast itself rounds
         t to the nearest integer, RNE — matching jnp.round)
  inv : psum[ro]  = sum_yin (W2 * q[p]*coef/255)^T @ q_yin         (7 fp16 matmuls)
  out : o[ro]     = psum + corr[p]                                  (DVE; corr folds
         the -1536 offset removal and the +128/255 bias)

Host post-processing un-blocks, upcasts and clips to [0,1]. Input is centered
(x - 0.5) on the host to halve fp16 quantization error; the DC shift this
introduces is folded into the round bias. Quant tables replicate the
reference's flattened-(b,c)<16 luma/chroma split, so tables vary per core.
All matmuls run fp16 (1 cyc/row); measured rel-L2 vs the f32 reference is
~6.5e-3, dominated by round()-flips from fp16 input/weight rounding.
"""

import numpy as np

import concourse.bass as bass
import concourse.mybir as mybir
import concourse.tile as tile
from concourse import bacc
from concourse.bass_utils import run_bass_kernel_spmd

N_CORES = 8
BS = 16
IMGS_PER_CORE = BS // N_CORES          # 2
SLICES = IMGS_PER_CORE * 3             # 6
MAGIC = 1536.0                         # fp16 round-to-nearest at ulp=1 for [1024,2048)

F16 = mybir.dt.float16
F32 = mybir.dt.float32
IDENT = mybir.ActivationFunctionType.Identity

_LUM = np.array([[16,11,10,16,24,40,51,61],[12,12,14,19,26,58,60,55],[14,13,16,24,40,57,69,56],[14,17,22,29,51,87,80,62],[18,22,37,56,68,109,103,77],[24,35,55,64,81,104,113,92],[49,64,78,87,103,121,120,101],[72,92,95,98,112,100,103,99]], np.float64)
_CHROM = np.array([[17,18,24,47,99,99,99,99],[18,21,26,66,99,99,99,99],[24,26,56,99,99,99,99,99],[47,66,99,99,99,99,99,99],[99,99,99,99,99,99,99,99],[99,99,99,99,99,99,99,99],[99,99,99,99,99,99,99,99],[99,99,99,99,99,99,99,99]], np.float64)
_WYCC = np.array([[0.299, 0.587, 0.114], [-0.1687, -0.3313, 0.5], [0.5, -0.4187, -0.0813]], np.float64)
_S4TERMS = [
    [(0, 1.0), (2, 1.402)],                       # r
    [(0, 1.0), (1, -0.34414), (2, -0.71414)],     # g
    [(0, 1.0), (1, 1.772)],                       # b
]
_NTERM = 7


def _dct_mat():
    k = np.arange(8)[:, None]
    n = np.arange(8)[None, :]
    norm = np.where(k == 0, np.sqrt(1.0 / 8.0), np.sqrt(2.0 / 8.0))
    return norm * np.cos(np.pi / 8.0 * (n + 0.5) * k)


def _qtables(quality):
    q = max(1, min(100, int(quality)))
    scale = 5000.0 / q if q < 50 else 200.0 - 2.0 * q
    tbs = np.stack([_LUM, _CHROM]) * np.float32(scale)
    return np.clip((tbs.astype(np.float32) + 50.0) / 100.0, 1.0, 255.0).astype(np.float64)


def _w2():
    K64 = np.kron(_dct_mat(), _dct_mat())    # [freq 8u+v, pix 8r+c]
    return np.kron(np.eye(2), K64)           # [128 freq, 128 pix]


def _fwd_weights():
    """fp16 lhsT [pix, freq] per (oc, ic), packed [128, 9*128]."""
    W2 = _w2()
    w = np.zeros((128, 9 * 128), np.float16)
    for oc in range(3):
        for ic in range(3):
            w[:, 128 * (3 * oc + ic):128 * (3 * oc + ic + 1)] = \
                (W2 * (255.0 * _WYCC[oc, ic])).T.astype(np.float16)
    return w


def _core_tables(quality, core, fwdw):
    """Per-core inverse weights + ACT scale/bias + inverse corr vectors."""
    W2 = _w2()
    qt = _qtables(quality)
    invw = np.zeros((128, 2 * _NTERM * 128), np.float16)
    scl = np.zeros((128, SLICES), np.float32)
    bia = np.zeros((128, SLICES), np.float32)
    cor = np.zeros((128, SLICES), np.float32)
    for im in range(IMGS_PER_CORE):
        q2 = []
        rq2 = []
        for ch in range(3):
            tab = qt[0] if (6 * core + 3 * im + ch) < 16 else qt[1]
            qv = np.concatenate([tab.reshape(64), tab.reshape(64)])
            q2.append(qv)
            rq2.append((1.0 / qv).astype(np.float32))
        for oc in range(3):
            scl[:, 3 * im + oc] = rq2[oc]
            # round bias: +MAGIC, Y level shift, and centering compensation
            dct_adj = np.zeros(128, np.float64)
            if oc == 0:
                dct_adj[0] += -1024.0
                dct_adj[64] += -1024.0
            for ic in range(3):
                w = fwdw[:, 128 * (3 * oc + ic):128 * (3 * oc + ic + 1)].astype(np.float64)
                dct_adj += 0.5 * w.sum(axis=0)   # A(0.5*ones) at each freq
            bia[:, 3 * im + oc] = (MAGIC + rq2[oc].astype(np.float64) * dct_adj).astype(np.float32)
        t = 0
        for ro in range(3):
            corr = np.zeros(128, np.float64)
            for (yin, coef) in _S4TERMS[ro]:
                st = (W2 * (q2[yin][:, None] * (coef / 255.0))).astype(np.float16)
                invw[:, 128 * (_NTERM * im + t):128 * (_NTERM * im + t + 1)] = st
                corr += -MAGIC * st.astype(np.float64).sum(axis=0)
                t += 1
            cor[:, 3 * im + ro] = (corr + 128.0 / 255.0).astype(np.float32)
    return invw, scl, bia, cor


def _block(x):
    """[n, 512, 512] f32 -> [n, 128, 2048] blocked fp16 (centered)."""
    n = x.shape[0]
    return np.ascontiguousarray(
        (x - np.float32(0.5)).reshape(n, 32, 2, 8, 64, 8)
        .transpose(0, 2, 3, 5, 1, 4).reshape(n, 128, 2048).astype(np.float16)
    )


def _unblock(y):
    """[n, 128, 2048] -> [n, 512, 512]."""
    n = y.shape[0]
    return y.reshape(n, 2, 8, 8, 32, 64).transpose(0, 4, 1, 2, 5, 3).reshape(n, 512, 512)


def _trace():
    nc = bacc.Bacc("TRN2", target_bir_lowering=False, debug=False)

    xin = nc.dram_tensor("xin", [SLICES, 128, 2048], F16, kind="ExternalInput").ap()
    fwdw_d = nc.dram_tensor("fwdw", [128, 9 * 128], F16, kind="ExternalInput").ap()
    invw_d = nc.dram_tensor("invw", [128, 2 * _NTERM * 128], F16, kind="ExternalInput").ap()
    # vec packs [scl | bia | cor] as [128, 18] f32
    vec_d = nc.dram_tensor("vec", [128, 3 * SLICES], F32, kind="ExternalInput").ap()
    xout = nc.dram_tensor("xout", [SLICES, 128, 2048], F16, kind="ExternalOutput").ap()

    with tile.TileContext(nc) as tc:
        with (
            tc.tile_pool(name="wts", bufs=1) as wp,
            tc.tile_pool(name="xp", bufs=1) as xp,
            tc.tile_pool(name="qp", bufs=1) as qp,
            tc.tile_pool(name="op", bufs=1) as op,
            tc.tile_pool(name="psA", bufs=4, space="PSUM") as psAp,
            tc.tile_pool(name="psB", bufs=4, space="PSUM") as psBp,
        ):
            fwdw = wp.tile([128, 9 * 128], F16, tag="fwdw")
            nc.gpsimd.dma_start(fwdw[:], fwdw_d)

            # PE p-state warmup: burn the ramp on dummy matmuls while the
            # first input chunks are still in flight.
            warm = wp.tile([128, 512], F16, tag="warm")
            nc.vector.memzero(warm[:])
            for _w in range(6):
                wps = psAp.tile([128, 512], F32, tag="psA", name="wps")
                nc.tensor.matmul(wps[:], warm[:, 0:128], warm[:], start=True, stop=True)

            # All input DMAs on the sync queue, ordered by first-use time.
            # Per-image inputs live in one [128, 3*2048] tile; one DMA per
            # column chunk covers all 3 channels (single HWDGE slot).
            xt = [None] * IMGS_PER_CORE
            invw = wp.tile([128, 2 * _NTERM * 128], F16, tag="invw")
            xt[0] = xp.tile([128, 3 * 2048], F16, tag="x0", name="x0")
            xt[1] = xp.tile([128, 3 * 2048], F16, tag="x1", name="x1")
            vec = wp.tile([128, 3 * SLICES], F32, tag="vec")

            def xdma(im, lo, hi):
                nc.sync.dma_start(
                    xt[im][:].rearrange("p (c n) -> p c n", c=3)[:, :, lo:hi],
                    xin[3 * im:3 * im + 3].rearrange("c p n -> p c n")[:, :, lo:hi],
                )

            xdma(0, 0, 256)
            xdma(0, 256, 512)
            nc.sync.dma_start(vec[:], vec_d)
            xdma(0, 512, 1024)
            nc.sync.dma_start(invw[:, 0:_NTERM * 128], invw_d[:, 0:_NTERM * 128])
            xdma(0, 1024, 1536)
            xdma(0, 1536, 2048)
            for s in range(4):
                xdma(1, 512 * s, 512 * (s + 1))
            nc.sync.dma_start(
                invw[:, _NTERM * 128:], invw_d[:, _NTERM * 128:]
            )

            qt_ = [[None] * 3 for _ in range(IMGS_PER_CORE)]
            ot_ = [None] * IMGS_PER_CORE
            _IOFF = {0: 0, 1: 2, 2: 5}   # term offset per ro in _S4TERMS

            def fwd(im, lo, hi):
                if lo == 0:
                    for c in range(3):
                        qt_[im][c] = qp.tile([128, 2048], F16, tag=f"q{im}_{c}", name=f"q{im}_{c}")
                # produce Y, Cr, Cb in that order: the inverse consumes Cr
                # before Cb (R = Y + c*Cr comes first)
                for oc in (0, 2, 1):
                    ps = psAp.tile([128, 512], F32, tag="psA")
                    n = hi - lo
                    for k in range(3):
                        nc.tensor.matmul(
                            ps[:, 0:n], fwdw[:, 128 * (3 * oc + k):128 * (3 * oc + k + 1)],
                            xt[im][:, 2048 * k + lo:2048 * k + hi],
                            start=(k == 0), stop=(k == 2),
                        )
                    sl = 3 * im + oc
                    nc.scalar.activation(
                        qt_[im][oc][:, lo:hi], ps[:, 0:n], IDENT,
                        bias=vec[:, SLICES + sl:SLICES + sl + 1],
                        scale=vec[:, sl:sl + 1],
                    )

            def inv(im, lo, hi, tail=False):
                if lo == 0:
                    ot_[im] = op.tile([128, 3 * 2048], F16, tag=f"o{im}", name=f"o{im}")
                n = hi - lo
                for ro in range(3):
                    terms = _S4TERMS[ro]
                    ps = psBp.tile([128, 512], F32, tag="psB")
                    for ti, (yin, _) in enumerate(terms):
                        t = _IOFF[ro] + ti
                        nc.tensor.matmul(
                            ps[:, 0:n], invw[:, 128 * (_NTERM * im + t):128 * (_NTERM * im + t + 1)],
                            qt_[im][yin][:, lo:hi],
                            start=(ti == 0), stop=(ti == len(terms) - 1),
                        )
                    sl = 3 * im + ro
                    dst = ot_[im][:, 2048 * ro + lo:2048 * ro + hi]
                    cv = vec[:, 2 * SLICES + sl:2 * SLICES + sl + 1]
                    if tail and ro == 1:
                        # spread tail writes across ACT and DVE
                        nc.scalar.activation(dst, ps[:, 0:n], IDENT, bias=cv, scale=1.0)
                    else:
                        nc.vector.tensor_scalar_add(dst, ps[:, 0:n], cv)
                # one DMA for this chunk across all 3 output channels
                nc.sync.dma_start(
                    xout[3 * im:3 * im + 3].rearrange("c p n -> p c n")[:, :, lo:hi],
                    ot_[im][:].rearrange("p (c n) -> p c n", c=3)[:, :, lo:hi],
                )

            fwd(0, 0, 256)
            fwd(0, 256, 512)
            fwd(0, 512, 1024)
            inv(0, 0, 512)
            fwd(0, 1024, 1536)
            inv(0, 512, 1024)
            fwd(0, 1536, 2048)
            inv(0, 1024, 1536)
            fwd(1, 0, 512)
            inv(0, 1536, 2048)
            fwd(1, 512, 1024)
            inv(1, 0, 512)
            fwd(1, 1024, 1536)
            inv(1, 512, 1024)
            fwd(1, 1536, 2048)
            inv(1, 1024, 1536)
            inv(1, 1536, 1792, tail=True)
            inv(1, 1792, 2048, tail=True)
    nc.compile()
    return nc


_COMPILED = None


def _get_compiled():
    global _COMPILED
    if _COMPILED is None:
        _COMPILED = _trace()
    return _COMPILED


def kernel(img, quality):
    img = np.ascontiguousarray(np.asarray(img, np.float32))
    quality = int(np.asarray(quality))
    nc = _get_compiled()

    fwdw = _fwd_weights()
    in_maps = []
    for core in range(N_CORES):
        invw, scl, bia, cor = _core_tables(quality, core, fwdw)
        shard = img[IMGS_PER_CORE * core:IMGS_PER_CORE * (core + 1)].reshape(SLICES, 512, 512)
        in_maps.append({
            "xin": _block(shard), "fwdw": fwdw, "invw": invw,
            "vec": np.ascontiguousarray(np.concatenate([scl, bia, cor], axis=1)),
        })

    res = run_bass_kernel_spmd(nc, in_maps, core_ids=list(range(N_CORES)))
    out = np.stack([
        _unblock(res.results[c]["xout"].astype(np.float32)) for c in range(N_CORES)
    ])
    return np.clip(out.reshape(BS, 3, 512, 512), 0.0, 1.0)


if __name__ == "__main__":
    rng = np.random.default_rng(0)
    x = rng.random((BS, 3, 512, 512), dtype=np.float32)
    y = kernel(x, 80)
    print("kernel ran:", y.shape, y.dtype, float(y.min()), float(y.max()))
